# revision 17
# baseline (speedup 1.0000x reference)
"""BernNet (nn_BernNet_9543417332146) Trainium2 kernel.

Reference computation:
    h = relu(x @ W1 + b1) @ W2 + b2                      (MLP head)
    out = sum_j  C(K,j)/2^K * relu(temp)_j * L^j (2I-L)^{K-j} h
  with L = I - A  (A = sym-normalized adjacency), evaluated by the
  reference via 65 sparse matvecs.

All terms are polynomials in A and commute, so
    out = p(A) h,   p(l) = sum_j c_j T_j (1-l)^j (1+l)^{K-j}
a degree-K polynomial whose coefficients depend only on `temp`.  For
temp = ones (the initialized BernNet parameters), the binomial sum
telescopes:  sum_j C(K,j) (1-l)^j (1+l)^{K-j} = 2^K  =>  p == 1, i.e.
the whole graph propagation is the identity and out == h exactly.

This kernel computes the polynomial coefficients from `temp` at runtime
with exact integer arithmetic, runs the MLP on all 8 NeuronCores
(nodes row-sharded, weights replicated), and only performs sparse
matvec work for the (never-initialized) case of nonzero higher-degree
coefficients, via a Horner evaluation needing deg(p) matvecs instead of
the reference's 65.

Device program (v2, 16-bit IO):
  - inputs stream in bf16 (x pre-cast on host), output stored fp16 and
    upcast on host; rel tolerance is 2e-2, bf16 path lands ~5e-3.
  - x packed transposed [128, HALF]: partitions 0..63 = features of the
    first HALF rows, 64..127 = features of the second HALF, so every DMA
    uses all 128 partitions contiguously.
  - mm1 runs the two 64-feature halves as row-tiled matmuls
    (tile_position rows 0/64, auto-derived from base partitions); mm2
    runs the two 64-feature output halves col-tiled into one PSUM bank
    (partitions 0-63 / 64-127), so PSUM evacuation runs at the full 128
    partition width.
  - PSUM tiles are [128, 2048] (4 banks); evacuation (relu+bias, or
    bias+cast for the output) is split between the ACT and DVE engines,
    which are the throughput bottleneck of this kernel (~1 elem/cycle/
    lane from fp32 PSUM).
  - loads on the SP HWDGE ring, stores on the ACT ring so they
    interleave at SDMA packet granularity instead of head-blocking.
"""

import numpy as np
from math import comb

N_NODES = 50000
FEATURES = 64
NHID = 128
NCORES = 8
ROWS_PER_CORE = 6272          # 8 * 6272 = 50176 >= 50000 (zero padded)
HALF = ROWS_PER_CORE // 2     # 3136 = 6*512 + 64
# per-half column chunks (start, width); <=512 so one fp32 PSUM bank each
CHUNKS = [(0, 512), (512, 512), (1024, 512), (1536, 512),
          (2048, 512), (2560, 512), (3072, 64)]
GROUPS = [[0, 1], [2, 3], [4, 5], [6]]   # phase-1 (mm1+relu) psum groups
YGROUPS = [[0, 1, 2, 3], [4, 5, 6]]      # phase-2 (mm2+bias) psum groups

# Blob column layout (constants first so the first DMA piece covers them).
# Biases travel in a separate tiny fp32 tensor ("bc"): the elementwise
# engines require fp32 scalar operands.
C_W1 = 0                      # [0,128)   W1 duplicated on both halves
C_W2 = NHID                   # [128,192) W2 (all 128 partitions)
C_X = C_W2 + FEATURES         # 192
BLOBW = C_X + HALF            # 3328

_nc_cache = {}


def _bern_poly_coefs(temp):
    """Coefficients a_m of p(A) = sum_m a_m A^m for the BernNet filter.

    p(l) = sum_j [C(K,j)/2^K] * relu(temp_j) * (1-l)^j (1+l)^{K-j}.
    The inner binomial products are exact integers, so for temp = ones
    the higher coefficients cancel to exactly 0.0 in float arithmetic.
    """
    k = temp.shape[0] - 1
    T = np.maximum(np.asarray(temp, np.float64), 0.0)
    a = np.zeros(k + 1)
    for j in range(k + 1):
        tj = T[j]
        if tj == 0.0:
            continue
        for m in range(k + 1):
            s = 0
            for p in range(max(0, m - (k - j)), min(j, m) + 1):
                s += (-1) ** p * comb(j, p) * comb(k - j, m - p)
            a[m] += (comb(k, j) * s) * tj / float(2**k)
    return a


def _build_mlp_nc(repeat=1, dbg_groups=4, dbg_ygroups=2):
    """SPMD per-core program: y = (relu(x@W1+b1))@W2+b2 for a 6272-row
    shard.  See module docstring for the dataflow.  dbg_* truncate the
    program for hardware bisection (outputs incomplete)."""
    import concourse.bass as bass
    import concourse.bacc as bacc
    import concourse.mybir as mybir
    from concourse.tile import TileContext

    f32 = mybir.dt.float32
    bf16 = mybir.dt.bfloat16
    fp16 = mybir.dt.float16
    relu = mybir.ActivationFunctionType.Relu
    ident = mybir.ActivationFunctionType.Identity
    add_op = mybir.AluOpType.add
    max_op = mybir.AluOpType.max
    # Bacc (not bare Bass): its lowering legalizes multi-wait instructions
    # into fused event-semaphore sequences the TRN2 encoders accept.
    nc = bacc.Bacc(None, target_bir_lowering=False)

    blob = nc.dram_tensor("blob", [128, BLOBW], bf16, kind="ExternalInput")
    bc = nc.dram_tensor("bc", [128, 2], f32, kind="ExternalInput")
    yt = nc.dram_tensor("yt", [128, HALF], fp16, kind="ExternalOutput")

    with TileContext(nc) as tc:
        with (
            tc.tile_pool(name="io", bufs=2) as iopool,
            tc.tile_pool(name="rt", bufs=8) as rtpool,
            tc.tile_pool(name="rt3", bufs=2) as rt3pool,
            tc.tile_pool(name="yc", bufs=2) as ycpool,
            tc.tile_pool(name="warmp", bufs=1) as warmpool,
            tc.tile_pool(name="psum", bufs=2, space=bass.MemorySpace.PSUM) as ppool,
        ):
            # Pre-warm the ACT function-table (LoadActFuncSet ~2.7us) and
            # the PE HAM clock before any data arrives.
            warm = warmpool.tile([1, 1], f32, tag="warm")
            nc.vector.memset(warm[:], 0.0)
            nc.scalar.activation(warm[:], warm[:], relu)
            scr = warmpool.tile([128, 384], bf16, tag="scr")
            nc.vector.memset(scr[:], 0.0)
            pw = ppool.tile([128, 2048], f32, tag="ps")
            for _ in range(3):
                nc.tensor.matmul(
                    pw[:, :256], scr[:64, :128], scr[:64, 128:384],
                    start=True, stop=True,
                )

            # repeat>1 re-runs the whole body (DMAs included) inside one
            # NEFF — used by the test harness to measure steady-state HW
            # time via (T(R2)-T(R1))/(R2-R1), cancelling dispatch overhead.
            for _rep in range(repeat):
                bt = iopool.tile([128, BLOBW], bf16, tag="blob")
                bct = iopool.tile([128, 2], f32, tag="bc")
                nc.sync.dma_start(bct[:], bc[:])
                pieces = ((0, C_X + 1024), (C_X + 1024, C_X + 2048),
                          (C_X + 2048, BLOBW))
                for p0c, p1c in pieces:
                    nc.sync.dma_start(bt[:, p0c:p1c], blob[:, p0c:p1c])

                w1h = [bt[0:64, C_W1:C_W1 + NHID],
                       bt[64:128, C_W1:C_W1 + NHID]]
                b1t = bct[:, 0:1]
                w2t = bt[:, C_W2:C_W2 + FEATURES]
                b2t = bct[:, 1:2]

                # ---- phase 1: mm1 (row-tiled halves) + relu ----
                rts = {}
                for gi, group in enumerate(GROUPS[:dbg_groups]):
                    ps = ppool.tile([128, 2048], f32, tag="ps")
                    if gi < 3:
                        rt = rtpool.tile([128, 2048], bf16, tag="rt")
                        ew = 2048
                        for k, c in enumerate(group):
                            c0, w = CHUNKS[c]
                            for h in range(2):
                                xs = bt[64 * h:64 * h + 64,
                                        C_X + c0:C_X + c0 + w]
                                nc.tensor.matmul(
                                    ps[:, k * 1024 + h * 512:
                                       k * 1024 + h * 512 + w],
                                    w1h[h], xs, start=True, stop=True,
                                )
                    else:
                        # tail: the two row-tiled mm1 halves run
                        # concurrently on the PE, so they must land in
                        # different PSUM banks (PE-W/PE-W same-bank
                        # collision is a hardware error).
                        rt = rt3pool.tile([128, 128], bf16, tag="rt3")
                        ew = None
                        c0, w = CHUNKS[group[0]]
                        for h in range(2):
                            xs = bt[64 * h:64 * h + 64,
                                    C_X + c0:C_X + c0 + w]
                            nc.tensor.matmul(
                                ps[:, h * 512:h * 512 + w],
                                w1h[h], xs, start=True, stop=True,
                            )
                    # evacuate: relu(z + b1), fp32 PSUM -> bf16 SBUF.
                    # ACT takes groups 0/2, DVE 1/3 (balanced makespan).
                    if ew is not None:
                        pairs = [(rt[:, :ew], ps[:, :ew])]
                    else:  # tail halves sit in separate banks
                        pairs = [(rt[:, 0:64], ps[:, 0:64]),
                                 (rt[:, 64:128], ps[:, 512:576])]
                    for rdst, psrc in pairs:
                        if gi % 2 == 0:
                            nc.scalar.activation(rdst, psrc, relu, bias=b1t)
                        else:
                            nc.vector.tensor_scalar(rdst, psrc, b1t, 0.0,
                                                    add_op, max_op)
                    rts[gi] = rt

                # ---- phase 2: mm2 (col-tiled halves) + bias + store ----
                for yi, ygroup in enumerate(YGROUPS[:dbg_ygroups]):
                    ps2 = ppool.tile([128, 2048], f32, tag="ps")
                    for k, c in enumerate(ygroup):
                        c0, w = CHUNKS[c]
                        gi, within = divmod(c, 2)
                        if c < 6:
                            rtsl = rts[gi][:, within * 1024:
                                           within * 1024 + 1024]
                            hoff = 512
                        else:
                            rtsl = rts[3]
                            hoff = 64
                        for h in range(2):
                            mov = rtsl[:, h * hoff:h * hoff + w]
                            nc.tensor.matmul(
                                ps2[64 * h:64 * h + 64,
                                    k * 512:k * 512 + w],
                                w2t, mov, start=True, stop=True,
                            )
                    ew = 2048 if yi == 0 else 1088
                    yc = ycpool.tile([128, ew], fp16, tag=f"yc{yi}")
                    # y + b2, fp32 PSUM -> fp16 SBUF. DVE takes Y0, ACT Y1.
                    if yi == 0:
                        nc.vector.tensor_scalar_add(yc[:, :ew],
                                                    ps2[:, :ew], b2t)
                    else:
                        nc.scalar.activation(yc[:, :ew], ps2[:, :ew],
                                             ident, bias=b2t)
                    nc.scalar.dma_start(
                        yt[:, yi * 2048:yi * 2048 + ew], yc[:, :ew])
    nc.compile()
    return nc


def _build_blobs(x, W1, b1, W2, b2):
    """Full inputs -> (bf16 blobs [NCORES, 128, BLOBW],
                       fp32 biases [NCORES, 128, 2])."""
    import ml_dtypes

    n = x.shape[0]
    n_pad = NCORES * ROWS_PER_CORE
    x_pad = np.zeros((n_pad, FEATURES), np.float32)
    x_pad[:n] = x
    consts = np.zeros((128, C_X), np.float32)
    consts[:, C_W1:C_W1 + NHID] = np.concatenate([W1, W1], axis=0)
    consts[:, C_W2:C_W2 + FEATURES] = W2

    blob_all = np.empty((NCORES, 128, BLOBW), np.float32)
    blob_all[:, :, :C_X] = consts
    blob_all[:, :, C_X:] = (
        x_pad.reshape(NCORES, 2, HALF, FEATURES)
        .transpose(0, 1, 3, 2)
        .reshape(NCORES, 128, HALF)
    )
    bc = np.stack(
        [b1.astype(np.float32), np.concatenate([b2, b2]).astype(np.float32)],
        axis=1,
    )  # [128, 2]
    bc_all = np.broadcast_to(bc, (NCORES, 128, 2)).copy()
    return blob_all.astype(ml_dtypes.bfloat16), bc_all


def _unpack_y(y_all, n=N_NODES):
    """[NCORES, 128, HALF] fp16 device output -> [n, 64] fp32 rows."""
    h = (
        np.asarray(y_all, np.float32)
        .reshape(NCORES, 2, FEATURES, HALF)
        .transpose(0, 1, 3, 2)
        .reshape(NCORES * ROWS_PER_CORE, FEATURES)
    )
    return np.ascontiguousarray(h[:n])


def _mlp_numpy(x, W1, b1, W2, b2):
    return np.maximum(x @ W1 + b1, 0.0) @ W2 + b2


def _make_runner(nc, n_cores=NCORES):
    """Persistent jitted executor for a prebuilt Bass module (mirrors
    bass2jax.run_bass_via_pjrt's sharded path, but jit-compiled once and
    without donation so it can be invoked repeatedly for timing).

    Returns (fn, in_names, out_names, out_avals): fn takes the
    axis-0-concatenated per-core inputs followed by concatenated zero
    output buffers and returns concatenated outputs.
    """
    import jax
    import concourse.mybir as mybir
    from concourse import bass2jax
    from jax.experimental.shard_map import shard_map
    from jax.sharding import Mesh, PartitionSpec

    bass2jax.install_neuronx_cc_hook()
    partition_name = nc.partition_id_tensor.name if nc.partition_id_tensor else None
    in_names, out_names, out_avals = [], [], []
    for alloc in nc.m.functions[0].allocations:
        if not isinstance(alloc, mybir.MemoryLocationSet):
            continue
        name = alloc.memorylocations[0].name
        if alloc.kind == "ExternalInput":
            if name != partition_name:
                in_names.append(name)
        elif alloc.kind == "ExternalOutput":
            out_names.append(name)
            out_avals.append(
                jax.core.ShapedArray(
                    tuple(alloc.tensor_shape), mybir.dt.np(alloc.dtype)
                )
            )
    n_params = len(in_names)
    all_in = list(in_names) + list(out_names)
    if partition_name is not None:
        all_in.append(partition_name)

    def _body(*args):
        operands = list(args)
        if partition_name is not None:
            operands.append(bass2jax.partition_id_tensor())
        return tuple(
            bass2jax._bass_exec_p.bind(
                *operands,
                out_avals=tuple(out_avals),
                in_names=tuple(all_in),
                out_names=tuple(out_names),
                lowering_input_output_aliases=(),
                sim_require_finite=True,
                sim_require_nnan=True,
                nc=nc,
            )
        )

    import numpy as _np

    devices = jax.devices()[:n_cores]
    mesh = Mesh(_np.asarray(devices), ("core",))
    nin = n_params + len(out_names)
    fn = jax.jit(
        shard_map(
            _body,
            mesh=mesh,
            in_specs=(PartitionSpec("core"),) * nin,
            out_specs=(PartitionSpec("core"),) * len(out_names),
            check_rep=False,
        ),
        keep_unused=True,
    )
    return fn, in_names, out_names, out_avals


def _mlp_trn(x, W1, b1, W2, b2, trace=False):
    """Run the MLP row-sharded across the 8 NeuronCores. Returns
    (h, exec_time_ns) — exec_time_ns is only populated when an NTFF
    profiling hook is available (trace=True); the test harness instead
    measures HW time via inner-repeat deltas.

    Uses a persistent jitted executable (cached across calls) so repeat
    kernel() invocations skip the XLA re-trace/re-compile that
    run_bass_kernel_spmd pays per call."""
    n = x.shape[0]
    if "nc" not in _nc_cache:
        _nc_cache["nc"] = _build_mlp_nc()
    nc = _nc_cache["nc"]

    if "runner" not in _nc_cache:
        _nc_cache["runner"] = _make_runner(nc)
    fn, in_names, out_names, out_avals = _nc_cache["runner"]
    assert in_names == ["blob", "bc"] and out_names == ["yt"]

    blobs, bcs = _build_blobs(x, W1, b1, W2, b2)
    concat_blob = blobs.reshape(NCORES * 128, BLOBW)
    concat_bc = bcs.reshape(NCORES * 128, 2)
    zeros = np.zeros((NCORES * 128, HALF), np.float16)
    outs = fn(concat_blob, concat_bc, zeros)
    y = np.asarray(outs[0]).reshape(NCORES, 128, HALF)
    return _unpack_y(y, n), None


def kernel(x, edge_index, W1, b1, W2, b2, temp):
    x = np.asarray(x, np.float32)
    W1 = np.asarray(W1, np.float32)
    b1 = np.asarray(b1, np.float32)
    W2 = np.asarray(W2, np.float32)
    b2 = np.asarray(b2, np.float32)
    temp = np.asarray(temp, np.float32)
    n = x.shape[0]

    a = _bern_poly_coefs(temp)

    if x.shape == (N_NODES, FEATURES) and W1.shape == (FEATURES, NHID):
        h = None
        for attempt in range(2):
            try:
                h, _ = _mlp_trn(x, W1, b1, W2, b2)
                break
            except Exception as e:  # infrastructure failure only
                print(f"WARNING: TRN MLP attempt {attempt} failed "
                      f"({type(e).__name__}: {e})")
        if h is None:  # stay correct even if the device is wedged
            print("WARNING: falling back to numpy MLP")
            h = _mlp_numpy(x, W1, b1, W2, b2)
    else:
        h = _mlp_numpy(x, W1, b1, W2, b2)

    deg = 0
    for m in range(len(a) - 1, 0, -1):
        if a[m] != 0.0:
            deg = m
            break

    if deg == 0:
        out = h if a[0] == 1.0 else a[0] * h
        return np.ascontiguousarray(out.astype(np.float32))

    # General path (temp != initialized ones): Horner with deg(p) sparse
    # matvecs. Unreachable for the shipped problem instance.
    src = np.asarray(edge_index[0], np.int64)
    dst = np.asarray(edge_index[1], np.int64)
    deg_out = np.bincount(src, minlength=n).astype(np.float32)
    dinv = np.where(deg_out > 0, 1.0 / np.sqrt(np.maximum(deg_out, 1.0)), 0.0).astype(
        np.float32
    )
    w_edge = (dinv[src] * dinv[dst]).astype(np.float32)

    try:
        from scipy.sparse import coo_matrix

        A = coo_matrix((w_edge, (dst, src)), shape=(n, n)).tocsr()
        anorm = lambda z: (A @ z).astype(np.float32)
    except ImportError:
        def anorm(z):
            out = np.zeros_like(z)
            np.add.at(out, dst, w_edge[:, None] * z[src])
            return out

    z = (a[deg] * h).astype(np.float32)
    for m in range(deg - 1, -1, -1):
        z = (anorm(z) + a[m] * h).astype(np.float32)
    return np.ascontiguousarray(z.astype(np.float32))


# revision 24
# speedup vs baseline: 1.9710x; 1.9710x over previous
"""BernNet (nn_BernNet_9543417332146) Trainium2 kernel.

Reference computation:
    h = relu(x @ W1 + b1) @ W2 + b2                      (MLP head)
    out = sum_j  C(K,j)/2^K * relu(temp)_j * L^j (2I-L)^{K-j} h
  with L = I - A  (A = sym-normalized adjacency), evaluated by the
  reference via 65 sparse matvecs.

All terms are polynomials in A and commute, so
    out = p(A) h,   p(l) = sum_j c_j T_j (1-l)^j (1+l)^{K-j}
a degree-K polynomial whose coefficients depend only on `temp`.  For
temp = ones (the initialized BernNet parameters), the binomial sum
telescopes:  sum_j C(K,j) (1-l)^j (1+l)^{K-j} = 2^K  =>  p == 1, i.e.
the whole graph propagation is the identity and out == h exactly.

This kernel computes the polynomial coefficients from `temp` at runtime
with exact integer arithmetic, runs the MLP on all 8 NeuronCores
(nodes row-sharded, weights replicated), and only performs sparse
matvec work for the (never-initialized) case of nonzero higher-degree
coefficients, via a Horner evaluation needing deg(p) matvecs instead of
the reference's 65.

Device program (v2, 16-bit IO):
  - inputs stream in bf16 (x pre-cast on host), output stored fp16 and
    upcast on host; rel tolerance is 2e-2, bf16 path lands ~5e-3.
  - x packed transposed [128, HALF]: partitions 0..63 = features of the
    first HALF rows, 64..127 = features of the second HALF, so every DMA
    uses all 128 partitions contiguously.
  - mm1 runs the two 64-feature halves as row-tiled matmuls
    (tile_position rows 0/64, auto-derived from base partitions); mm2
    runs the two 64-feature output halves col-tiled into one PSUM bank
    (partitions 0-63 / 64-127), so PSUM evacuation runs at the full 128
    partition width.
  - PSUM tiles are [128, 2048] (4 banks); evacuation (relu+bias, or
    bias+cast for the output) is split between the ACT and DVE engines,
    which are the throughput bottleneck of this kernel (~1 elem/cycle/
    lane from fp32 PSUM).
  - loads on the SP HWDGE ring, stores on the ACT ring so they
    interleave at SDMA packet granularity instead of head-blocking.
"""

import numpy as np
from math import comb

N_NODES = 50000
FEATURES = 64
NHID = 128
NCORES = 8
ROWS_PER_CORE = 6272          # 8 * 6272 = 50176 >= 50000 (zero padded)
HALF = ROWS_PER_CORE // 2     # 3136 = 6*512 + 64
# per-half column chunks (start, width); <=512 so one fp32 PSUM bank each
CHUNKS = [(0, 512), (512, 512), (1024, 512), (1536, 512),
          (2048, 512), (2560, 512), (3072, 64)]
YGROUPS = [[0, 1], [2, 3], [4, 5], [6]]  # phase-2 (mm2+bias) psum groups

# Blob column layout (constants first so the first DMA piece covers them).
# b1/b2 are raw fp32 bits spanning 2 bf16 columns each (the elementwise
# engines need fp32 scalar operands; the device bitcasts them back — the
# runner's sim NaN-check is disabled since fp32 halfwords can alias bf16
# NaN patterns).
C_W1 = 0                      # [0,128)   W1 duplicated on both halves
C_B1 = NHID                   # [128,130) b1 per-partition (fp32 bits)
C_W2 = C_B1 + 2               # [130,194) W2 (all 128 partitions)
C_B2 = C_W2 + FEATURES        # [194,196) b2 duplicated (fp32 bits)
C_X = C_B2 + 2                # 196
BLOBW = C_X + HALF            # 3332

_nc_cache = {}


def _bern_poly_coefs(temp):
    """Coefficients a_m of p(A) = sum_m a_m A^m for the BernNet filter.

    p(l) = sum_j [C(K,j)/2^K] * relu(temp_j) * (1-l)^j (1+l)^{K-j}.
    The inner binomial products are exact integers, so for temp = ones
    the higher coefficients cancel to exactly 0.0 in float arithmetic.
    """
    k = temp.shape[0] - 1
    T = np.maximum(np.asarray(temp, np.float64), 0.0)
    a = np.zeros(k + 1)
    for j in range(k + 1):
        tj = T[j]
        if tj == 0.0:
            continue
        for m in range(k + 1):
            s = 0
            for p in range(max(0, m - (k - j)), min(j, m) + 1):
                s += (-1) ** p * comb(j, p) * comb(k - j, m - p)
            a[m] += (comb(k, j) * s) * tj / float(2**k)
    return a


def _build_mlp_nc(repeat=1, dbg_groups=7, dbg_ygroups=4):
    """SPMD per-core program: y = (relu(x@W1+b1))@W2+b2 for a 6272-row
    shard.  See module docstring for the dataflow.  dbg_* truncate the
    program for hardware bisection (outputs incomplete)."""
    import concourse.bass as bass
    import concourse.bacc as bacc
    import concourse.mybir as mybir
    from concourse.tile import TileContext

    f32 = mybir.dt.float32
    bf16 = mybir.dt.bfloat16
    fp16 = mybir.dt.float16
    relu = mybir.ActivationFunctionType.Relu
    ident = mybir.ActivationFunctionType.Identity
    add_op = mybir.AluOpType.add
    max_op = mybir.AluOpType.max
    # Bacc (not bare Bass): its lowering legalizes multi-wait instructions
    # into fused event-semaphore sequences the TRN2 encoders accept.
    nc = bacc.Bacc(None, target_bir_lowering=False)

    blob = nc.dram_tensor("blob", [128, BLOBW], bf16, kind="ExternalInput")
    yt = nc.dram_tensor("yt", [128, HALF], fp16, kind="ExternalOutput")

    with TileContext(nc) as tc:
        with (
            tc.tile_pool(name="io", bufs=2) as iopool,
            tc.tile_pool(name="rt", bufs=8) as rtpool,
            tc.tile_pool(name="rt3", bufs=2) as rt3pool,
            tc.tile_pool(name="yc", bufs=2) as ycpool,
            tc.tile_pool(name="warmp", bufs=1) as warmpool,
            tc.tile_pool(name="psum", bufs=4, space=bass.MemorySpace.PSUM) as ppool,
        ):
            # Pre-warm the ACT function-table (LoadActFuncSet ~2.7us) and
            # the PE HAM clock before any data arrives.
            warm = warmpool.tile([1, 1], f32, tag="warm")
            nc.vector.memset(warm[:], 0.0)
            nc.scalar.activation(warm[:], warm[:], relu)
            scr = warmpool.tile([128, 384], bf16, tag="scr")
            nc.vector.memset(scr[:], 0.0)
            pw = ppool.tile([128, 1024], f32, tag="ps")
            for _ in range(3):
                nc.tensor.matmul(
                    pw[:, :256], scr[:64, :128], scr[:64, 128:384],
                    start=True, stop=True,
                )

            # Evacuation engine assignment (balanced makespan: ACT is a
            # bit faster per op, so it takes one more big op than DVE).
            ACT_OPS = {"P0", "P2", "P4", "Y0", "Y2", "Y3"}

            def evac(key, dst, src, is_relu):
                if key in ACT_OPS:
                    nc.scalar.activation(dst, src, relu if is_relu else ident,
                                         bias=b1t if is_relu else b2t)
                elif is_relu:
                    nc.vector.tensor_scalar(dst, src, b1t, 0.0, add_op, max_op)
                else:
                    nc.vector.tensor_scalar_add(dst, src, b2t)

            # repeat>1 re-runs the whole body (DMAs included) inside one
            # NEFF — used by the test harness to measure steady-state HW
            # time via (T(R2)-T(R1))/(R2-R1), cancelling dispatch overhead.
            for _rep in range(repeat):
                bt = iopool.tile([128, BLOBW], bf16, tag="blob")
                pieces = ((0, C_X + 1024), (C_X + 1024, C_X + 2048),
                          (C_X + 2048, BLOBW))
                for p0c, p1c in pieces:
                    nc.sync.dma_start(bt[:, p0c:p1c], blob[:, p0c:p1c])

                w1h = [bt[0:64, C_W1:C_W1 + NHID],
                       bt[64:128, C_W1:C_W1 + NHID]]
                b1t = bt[:, C_B1:C_B1 + 2].bitcast(f32)
                w2t = bt[:, C_W2:C_W2 + FEATURES]
                b2t = bt[:, C_B2:C_B2 + 2].bitcast(f32)

                # ---- phase 1: mm1 (row-tiled halves) + relu ----
                # One [128,1024] psum tile per chunk: h0 in bank 0,
                # h1 in bank 1 (the row-tiled halves run concurrently on
                # the PE, so they must land in different banks:
                # PE-W/PE-W same-bank collision is a hardware error).
                rts = {}
                for ci in range(dbg_groups):
                    c0, w = CHUNKS[ci]
                    ps = ppool.tile([128, 1024], f32, tag="ps")
                    for h in range(2):
                        xs = bt[64 * h:64 * h + 64, C_X + c0:C_X + c0 + w]
                        nc.tensor.matmul(ps[:, h * 512:h * 512 + w],
                                         w1h[h], xs, start=True, stop=True)
                    # evacuate: relu(z + b1), fp32 PSUM -> bf16 SBUF
                    if ci < 6:
                        rt = rtpool.tile([128, 1024], bf16, tag="rt")
                        evac(f"P{ci}", rt[:, :1024], ps[:, :1024], True)
                    else:
                        rt = rt3pool.tile([128, 128], bf16, tag="rt3")
                        evac("P6", rt[:, 0:64], ps[:, 0:64], True)
                        evac("P6", rt[:, 64:128], ps[:, 512:576], True)
                    rts[ci] = rt

                # ---- phase 2: mm2 (col-tiled halves) + bias ----
                yc = ycpool.tile([128, HALF], fp16, tag="yc")
                for yi, ygroup in enumerate(YGROUPS[:dbg_ygroups]):
                    ps2 = ppool.tile([128, 1024], f32, tag="ps")
                    for k, c in enumerate(ygroup):
                        c0, w = CHUNKS[c]
                        hoff = 512 if c < 6 else 64
                        for h in range(2):
                            mov = rts[c][:, h * hoff:h * hoff + w]
                            nc.tensor.matmul(
                                ps2[64 * h:64 * h + 64, k * 512:k * 512 + w],
                                w2t, mov, start=True, stop=True,
                            )
                    # y + b2, fp32 PSUM -> fp16 SBUF (one shared yc tile;
                    # disjoint slices so the engines can run in parallel)
                    ybase = CHUNKS[ygroup[0]][0]
                    ew = sum(w for _, w in (CHUNKS[c] for c in ygroup))
                    evac(f"Y{yi}", yc[:, ybase:ybase + ew], ps2[:, :ew], False)
                # single output store from the otherwise-idle GPSIMD
                # (SWDGE), keeping the SP/ACT HWDGE rings for loads/compute
                nc.gpsimd.dma_start(yt[:, :], yc[:, :])
    nc.compile()
    return nc


def _build_blobs(x, W1, b1, W2, b2):
    """Full inputs -> bf16 blobs [NCORES, 128, BLOBW] (biases embedded as
    raw fp32 bits across 2 bf16 columns each)."""
    import ml_dtypes

    n = x.shape[0]
    n_pad = NCORES * ROWS_PER_CORE
    x_pad = np.zeros((n_pad, FEATURES), np.float32)
    x_pad[:n] = x
    consts = np.zeros((128, C_X), np.float32)
    consts[:, C_W1:C_W1 + NHID] = np.concatenate([W1, W1], axis=0)
    consts[:, C_W2:C_W2 + FEATURES] = W2

    blob_all = np.empty((NCORES, 128, BLOBW), np.float32)
    blob_all[:, :, :C_X] = consts
    blob_all[:, :, C_X:] = (
        x_pad.reshape(NCORES, 2, HALF, FEATURES)
        .transpose(0, 1, 3, 2)
        .reshape(NCORES, 128, HALF)
    )
    out = blob_all.astype(ml_dtypes.bfloat16)
    u16 = out.view(np.uint16)
    u16[:, :, C_B1:C_B1 + 2] = b1.astype(np.float32).view(np.uint16).reshape(128, 2)
    u16[:, :, C_B2:C_B2 + 2] = (
        np.concatenate([b2, b2]).astype(np.float32).view(np.uint16).reshape(128, 2)
    )
    return out


def _unpack_y(y_all, n=N_NODES):
    """[NCORES, 128, HALF] fp16 device output -> [n, 64] fp32 rows."""
    h = (
        np.asarray(y_all, np.float32)
        .reshape(NCORES, 2, FEATURES, HALF)
        .transpose(0, 1, 3, 2)
        .reshape(NCORES * ROWS_PER_CORE, FEATURES)
    )
    return np.ascontiguousarray(h[:n])


def _mlp_numpy(x, W1, b1, W2, b2):
    return np.maximum(x @ W1 + b1, 0.0) @ W2 + b2


def _make_runner(nc, n_cores=NCORES):
    """Persistent jitted executor for a prebuilt Bass module (mirrors
    bass2jax.run_bass_via_pjrt's sharded path, but jit-compiled once and
    without donation so it can be invoked repeatedly for timing).

    Returns (fn, in_names, out_names, out_avals): fn takes the
    axis-0-concatenated per-core inputs followed by concatenated zero
    output buffers and returns concatenated outputs.
    """
    import jax
    import concourse.mybir as mybir
    from concourse import bass2jax
    from jax.experimental.shard_map import shard_map
    from jax.sharding import Mesh, PartitionSpec

    bass2jax.install_neuronx_cc_hook()
    partition_name = nc.partition_id_tensor.name if nc.partition_id_tensor else None
    in_names, out_names, out_avals = [], [], []
    for alloc in nc.m.functions[0].allocations:
        if not isinstance(alloc, mybir.MemoryLocationSet):
            continue
        name = alloc.memorylocations[0].name
        if alloc.kind == "ExternalInput":
            if name != partition_name:
                in_names.append(name)
        elif alloc.kind == "ExternalOutput":
            out_names.append(name)
            out_avals.append(
                jax.core.ShapedArray(
                    tuple(alloc.tensor_shape), mybir.dt.np(alloc.dtype)
                )
            )
    n_params = len(in_names)
    all_in = list(in_names) + list(out_names)
    if partition_name is not None:
        all_in.append(partition_name)

    def _body(*args):
        operands = list(args)
        if partition_name is not None:
            operands.append(bass2jax.partition_id_tensor())
        return tuple(
            bass2jax._bass_exec_p.bind(
                *operands,
                out_avals=tuple(out_avals),
                in_names=tuple(all_in),
                out_names=tuple(out_names),
                lowering_input_output_aliases=(),
                # fp32 bias bits embedded in the bf16 blob can alias NaN
                # bf16 patterns — the value checks would false-positive
                sim_require_finite=False,
                sim_require_nnan=False,
                nc=nc,
            )
        )

    import numpy as _np

    devices = jax.devices()[:n_cores]
    mesh = Mesh(_np.asarray(devices), ("core",))
    nin = n_params + len(out_names)
    fn = jax.jit(
        shard_map(
            _body,
            mesh=mesh,
            in_specs=(PartitionSpec("core"),) * nin,
            out_specs=(PartitionSpec("core"),) * len(out_names),
            check_rep=False,
        ),
        keep_unused=True,
    )
    return fn, in_names, out_names, out_avals


def _mlp_trn(x, W1, b1, W2, b2, trace=False):
    """Run the MLP row-sharded across the 8 NeuronCores. Returns
    (h, exec_time_ns) — exec_time_ns is only populated when an NTFF
    profiling hook is available (trace=True); the test harness instead
    measures HW time via inner-repeat deltas.

    Uses a persistent jitted executable (cached across calls) so repeat
    kernel() invocations skip the XLA re-trace/re-compile that
    run_bass_kernel_spmd pays per call."""
    n = x.shape[0]
    if "nc" not in _nc_cache:
        _nc_cache["nc"] = _build_mlp_nc()
    nc = _nc_cache["nc"]

    if "runner" not in _nc_cache:
        _nc_cache["runner"] = _make_runner(nc)
    fn, in_names, out_names, out_avals = _nc_cache["runner"]
    assert in_names == ["blob"] and out_names == ["yt"]

    concat_blob = _build_blobs(x, W1, b1, W2, b2).reshape(NCORES * 128, BLOBW)
    zeros = np.zeros((NCORES * 128, HALF), np.float16)
    outs = fn(concat_blob, zeros)
    y = np.asarray(outs[0]).reshape(NCORES, 128, HALF)
    return _unpack_y(y, n), None


def kernel(x, edge_index, W1, b1, W2, b2, temp):
    x = np.asarray(x, np.float32)
    W1 = np.asarray(W1, np.float32)
    b1 = np.asarray(b1, np.float32)
    W2 = np.asarray(W2, np.float32)
    b2 = np.asarray(b2, np.float32)
    temp = np.asarray(temp, np.float32)
    n = x.shape[0]

    a = _bern_poly_coefs(temp)

    if x.shape == (N_NODES, FEATURES) and W1.shape == (FEATURES, NHID):
        h = None
        for attempt in range(2):
            try:
                h, _ = _mlp_trn(x, W1, b1, W2, b2)
                break
            except Exception as e:  # infrastructure failure only
                print(f"WARNING: TRN MLP attempt {attempt} failed "
                      f"({type(e).__name__}: {e})")
        if h is None:  # stay correct even if the device is wedged
            print("WARNING: falling back to numpy MLP")
            h = _mlp_numpy(x, W1, b1, W2, b2)
    else:
        h = _mlp_numpy(x, W1, b1, W2, b2)

    deg = 0
    for m in range(len(a) - 1, 0, -1):
        if a[m] != 0.0:
            deg = m
            break

    if deg == 0:
        out = h if a[0] == 1.0 else a[0] * h
        return np.ascontiguousarray(out.astype(np.float32))

    # General path (temp != initialized ones): Horner with deg(p) sparse
    # matvecs. Unreachable for the shipped problem instance.
    src = np.asarray(edge_index[0], np.int64)
    dst = np.asarray(edge_index[1], np.int64)
    deg_out = np.bincount(src, minlength=n).astype(np.float32)
    dinv = np.where(deg_out > 0, 1.0 / np.sqrt(np.maximum(deg_out, 1.0)), 0.0).astype(
        np.float32
    )
    w_edge = (dinv[src] * dinv[dst]).astype(np.float32)

    try:
        from scipy.sparse import coo_matrix

        A = coo_matrix((w_edge, (dst, src)), shape=(n, n)).tocsr()
        anorm = lambda z: (A @ z).astype(np.float32)
    except ImportError:
        def anorm(z):
            out = np.zeros_like(z)
            np.add.at(out, dst, w_edge[:, None] * z[src])
            return out

    z = (a[deg] * h).astype(np.float32)
    for m in range(deg - 1, -1, -1):
        z = (anorm(z) + a[m] * h).astype(np.float32)
    return np.ascontiguousarray(z.astype(np.float32))
